# revision 8
# baseline (speedup 1.0000x reference)
"""Causal single-head attention on 8 trn2 NeuronCores.

Sharding: batch b = core//2, pair-half h = core%2. Each batch's 4096 queries
split into 4 chunks of 1024; h=0 owns chunks {0,3}, h=1 owns {1,2} (balanced
causal work). Host pre-transposes x to xT[1024, T] and permutes key chunks
per-core (layout [ownA, ownB, o1, o2]) so one uniform SPMD program runs on all
cores; per-core behavior differs only through input data (chunk order + a tiny
per-pass bias table that turns never-needed key chunks off via exp(s - 80)).

Device dataflow (per core, all in transposed orientation):
  kvT[128, 4096] = [Wk|Wv]^T @ xT      (fused projection, full PE array)
  qT [64, 2048]  = (Wq/32)^T @ xT[:, own]
  S^T[keys,q] blocks = kT^T-slices @ qT  (f32r matmuls, causal blocks only)
  exp on ACT (PSUM->SBUF) with additive bias; staircase masks via affine_select
  outT[65, q] += [v|1]^T-tiles @ exp     (row 64 = softmax denominator, free)
  out = transpose(outT) * recip(sums), DMA to DRAM.
"""

import sys

if "/opt/trn_rl_repo" not in sys.path:
    sys.path.insert(0, "/opt/trn_rl_repo")

import numpy as np

B, T, D, DK = 4, 4096, 1024, 64
C = 1024          # T-chunk size (4 chunks per batch)
NEG = -80.0       # additive bias for masked chunks: exp(s-80) ~ 1e-35
N_CORES = 8

_CACHE = {}


def _build_nc():
    from contextlib import ExitStack

    import concourse.bass as bass  # noqa: F401
    import concourse.mybir as mybir
    import concourse.tile as tile
    from concourse import bacc
    from concourse.masks import make_identity

    f32 = mybir.dt.float32
    f32r = mybir.dt.float32r
    Exp = mybir.ActivationFunctionType.Exp

    nc = bacc.Bacc("TRN2", target_bir_lowering=False, debug=False,
                   num_devices=N_CORES)

    xt_d = nc.dram_tensor("xt", [D, T], f32, kind="ExternalInput").ap()
    wq_d = nc.dram_tensor("wq", [D, DK], f32, kind="ExternalInput").ap()
    wkv_d = nc.dram_tensor("wkv", [D, 2 * DK], f32, kind="ExternalInput").ap()
    pb_d = nc.dram_tensor("pbias", [128, 6], f32, kind="ExternalInput").ap()
    out_d = nc.dram_tensor("out", [2 * C, DK], f32, kind="ExternalOutput").ap()

    # pass table: (q-chunk sel, key layout position, diag?, bias column)
    # execution order: qA passes first (2 live outT banks), then qB.
    PASSES = [
        (0, 0, True, None),   # p0: qA vs its own chunk (diagonal)
        (0, 2, False, 1),     # p1: qA vs layout pos 2 (bias: full or off)
        (1, 0, False, 2),     # p2: qB vs pos 0
        (1, 1, True, None),   # p3: qB vs its own chunk (diagonal)
        (1, 2, False, 4),     # p4
        (1, 3, False, 5),     # p5
    ]
    # DMA / projection order of layout chunks: qA needs 0 and 2 first.
    CHUNK_ORDER = [0, 2, 1, 3]

    with tile.TileContext(nc) as tc, ExitStack() as ctx:
        wpool = ctx.enter_context(tc.tile_pool(name="w", bufs=1))
        xtpool = ctx.enter_context(tc.tile_pool(name="xt", bufs=16))
        kvpool = ctx.enter_context(tc.tile_pool(name="kv", bufs=1))
        exppool = ctx.enter_context(tc.tile_pool(name="exp", bufs=3))
        outsb = ctx.enter_context(tc.tile_pool(name="outsb", bufs=2))
        opool = ctx.enter_context(tc.tile_pool(name="o", bufs=4))
        scps = ctx.enter_context(tc.tile_pool(name="scps", bufs=1, space="PSUM"))
        accps = ctx.enter_context(tc.tile_pool(name="accps", bufs=2, space="PSUM"))
        prps = ctx.enter_context(tc.tile_pool(name="prps", bufs=2, space="PSUM"))

        # ---- constants / weights ----
        ident = wpool.tile([128, 128], f32)
        make_identity(nc, ident[:])
        ident_r = wpool.tile([128, 128], f32r)
        nc.vector.tensor_copy(ident_r[:], ident[:])
        wkv_s = wpool.tile([128, 8, 2 * DK], f32r)
        nc.sync.dma_start(wkv_s[:], wkv_d.rearrange("(dc p) m -> p dc m", p=128).bitcast(f32r))
        wq_s = wpool.tile([128, 8, DK], f32r)
        nc.sync.dma_start(wq_s[:], wq_d.rearrange("(dc p) m -> p dc m", p=128).bitcast(f32r))
        pb_s = wpool.tile([128, 6], f32)
        nc.sync.dma_start(pb_s[:], pb_d[:])

        # ---- persistent activations ----
        kvT = kvpool.tile([128, T], f32r)          # rows 0:64 kT, 64:128 vT
        qT = kvpool.tile([64, 2 * C], f32r)
        vall = kvpool.tile([128, 32, DK + 1], f32r)  # [keys, kt, v|1]
        ones_sc = wpool.tile([128, 32], f32)
        nc.vector.memset(ones_sc[:], 1.0)
        nc.vector.tensor_copy(
            vall[:, :, DK : DK + 1],
            ones_sc[:].rearrange("p (a b) -> p a b", b=1),
        )

        # ---- load xt + projections, chunk by chunk ----
        for lc in CHUNK_ORDER:
            xts = []
            for dc in range(8):
                t = xtpool.tile([128, C], f32r, tag="xt")
                nc.sync.dma_start(
                    t[:], xt_d[dc * 128 : (dc + 1) * 128, lc * C : (lc + 1) * C].bitcast(f32r)
                )
                xts.append(t)
            for g in range(2):  # 512-column groups within the chunk
                sl = slice(g * 512, (g + 1) * 512)
                kv_ps = prps.tile([128, 512], f32, tag="pr")
                for dc in range(8):
                    nc.tensor.matmul(
                        kv_ps[:],
                        lhsT=wkv_s[:, dc, :],
                        rhs=xts[dc][:, sl],
                        start=(dc == 0), stop=(dc == 7),
                    )
                nc.vector.tensor_copy(kvT[:, lc * C + g * 512 : lc * C + (g + 1) * 512],
                                      kv_ps[:])
                if lc < 2:  # q projection for own chunks (layout pos 0, 1)
                    q_ps = prps.tile([64, 512], f32, tag="pr")
                    for dc in range(8):
                        nc.tensor.matmul(
                            q_ps[:],
                            lhsT=wq_s[:, dc, :],
                            rhs=xts[dc][:, sl],
                            start=(dc == 0), stop=(dc == 7),
                        )
                    nc.vector.tensor_copy(
                        qT[:, lc * C + g * 512 : lc * C + (g + 1) * 512], q_ps[:]
                    )
            # v' tiles: transpose vT[64, 128] -> [128, 64] per key tile
            for ktl in range(8):
                kt = lc * 8 + ktl
                tr_ps = prps.tile([128, DK], f32r, tag="pr", name="tr_ps")
                nc.tensor.transpose(
                    tr_ps[:],
                    kvT[64:128, kt * 128 : (kt + 1) * 128],
                    ident_r[64:128, 64:128],
                )
                nc.vector.tensor_copy(vall[:, kt, 0:DK], tr_ps[:])

        # ---- attention passes ----
        acc = {}       # qg (0..3) -> psum accumulator [65, 512]
        mm_done = {}   # qg -> matmuls issued into acc
        MM_TOTAL = {0: 12, 1: 16, 2: 28, 3: 32}

        def drain(qg):
            ot = outsb.tile([65, 512], f32, tag="ot")
            nc.vector.tensor_copy(ot[:], acc[qg][:])
            for j in range(4):
                tp = prps.tile([128, 65], f32, tag="pr")
                nc.tensor.transpose(
                    tp[:],
                    ot[:, j * 128 : (j + 1) * 128],
                    ident[0:65, 0:65],
                )
                rec = opool.tile([128, 1], f32, tag="rec")
                nc.vector.reciprocal(rec[:], tp[:, DK : DK + 1])
                ob = opool.tile([128, DK], f32, tag="ob")
                nc.vector.tensor_scalar_mul(ob[:], tp[:, 0:DK], rec[:])
                r0 = qg * 512 + j * 128
                nc.sync.dma_start(out_d[r0 : r0 + 128, :], ob[:])

        for pi, (qsel, kp, diag, bcol) in enumerate(PASSES):
            for qg_l in range(2):
                qg = qsel * 2 + qg_l
                if qg not in acc:
                    acc[qg] = accps.tile([65, 512], f32, tag="acc",
                                         name=f"acc{qg}")
                    mm_done[qg] = 0
                qc0 = qsel * 1024 + qg_l * 512
                # substeps of 4 key tiles; diag passes skip kt>=4 for qg_l==0
                n_kt = 4 if (diag and qg_l == 0) else 8
                for s0 in range(0, n_kt, 4):
                    sc = scps.tile([128, 2048], f32, tag="sc")
                    for i in range(4):
                        ktl = s0 + i
                        kc0 = kp * 1024 + ktl * 128
                        nc.tensor.matmul(
                            sc[:, i * 512 : (i + 1) * 512],
                            lhsT=kvT[0:64, kc0 : kc0 + 128],
                            rhs=qT[:, qc0 : qc0 + 512],
                            start=True, stop=True,
                        )
                    et = exppool.tile([128, 2048], f32r, tag="et")
                    bias = pb_s[:, bcol : bcol + 1] if bcol is not None else 0.0
                    nc.scalar.activation(et[:], sc[:], Exp, bias=bias)
                    if diag:
                        for i in range(4):
                            ktl = s0 + i
                            m = ktl - 4 * qg_l
                            if 0 <= m < 4:
                                # keep exp where key<=q: j - p - 128m >= 0
                                nc.gpsimd.affine_select(
                                    out=et[:, i * 512 : (i + 1) * 512],
                                    in_=et[:, i * 512 : (i + 1) * 512],
                                    compare_op=mybir.AluOpType.is_ge,
                                    fill=0.0,
                                    base=-(128 * m),
                                    channel_multiplier=-1,
                                    pattern=[[1, 512]],
                                )
                    for i in range(4):
                        ktl = s0 + i
                        kt = kp * 8 + ktl
                        nc.tensor.matmul(
                            acc[qg][:],
                            lhsT=vall[:, kt, :],
                            rhs=et[:, i * 512 : (i + 1) * 512],
                            start=(mm_done[qg] == 0),
                            stop=(mm_done[qg] == MM_TOTAL[qg] - 1),
                        )
                        mm_done[qg] += 1
            if pi == 1:
                drain(0), drain(1)
            if pi == 5:
                drain(2), drain(3)

    nc.compile()
    return nc


def get_nc():
    if "nc" not in _CACHE:
        _CACHE["nc"] = _build_nc()
    return _CACHE["nc"]


def make_in_maps(x, Wq, Wk, Wv):
    wq_s = np.ascontiguousarray(Wq.astype(np.float32) / 32.0)
    wkv = np.ascontiguousarray(
        np.concatenate([Wk, Wv], axis=1).astype(np.float32)
    )
    in_maps = []
    for core in range(N_CORES):
        b, h = core // 2, core % 2
        order = [0, 3, 1, 2] if h == 0 else [1, 2, 0, 3]
        xbt = x[b].T  # [D, T] view
        xt = np.ascontiguousarray(
            np.concatenate([xbt[:, c * C : (c + 1) * C] for c in order], axis=1),
            dtype=np.float32,
        )
        bias_vals = [0, NEG, 0, 0, 0, 0] if h == 0 else [0, 0, 0, 0, 0, NEG]
        pb = np.ascontiguousarray(
            np.broadcast_to(np.array(bias_vals, np.float32), (128, 6))
        )
        in_maps.append({"xt": xt, "wq": wq_s, "wkv": wkv, "pbias": pb})
    return in_maps


def gather_out(results):
    out = np.empty((B, T, DK), np.float32)
    for core in range(N_CORES):
        b, h = core // 2, core % 2
        cA, cB = (0, 3) if h == 0 else (1, 2)
        o = results[core]["out"]
        out[b, cA * C : (cA + 1) * C] = o[0:C]
        out[b, cB * C : (cB + 1) * C] = o[C : 2 * C]
    return out


def run(in_maps, trace=False, tmpdir=None):
    from concourse.bass_utils import run_bass_kernel_spmd

    nc = get_nc()
    return run_bass_kernel_spmd(
        nc, in_maps, core_ids=list(range(N_CORES)), trace=trace, tmpdir=tmpdir
    )


def kernel(x, Wq, Wk, Wv):
    x = np.asarray(x, dtype=np.float32)
    in_maps = make_in_maps(x, np.asarray(Wq), np.asarray(Wk), np.asarray(Wv))
    res = run(in_maps)
    return gather_out(res.results)


# revision 11
# speedup vs baseline: 1.0449x; 1.0449x over previous
"""Causal single-head attention on 8 trn2 NeuronCores.

Sharding: batch b = core//2, pair-half h = core%2. Each batch's 4096 queries
split into 4 chunks of 1024; h=0 owns chunks {0,3}, h=1 owns {1,2} (balanced
causal work). Host pre-transposes x to xT[1024, T] and permutes key chunks
per-core (layout [ownA, ownB, o1, o2]) so one uniform SPMD program runs on all
cores; per-core behavior differs only through input data (chunk order + a tiny
per-pass bias table that turns never-needed key chunks off via exp(s - 80)).

Device dataflow (per core, all in transposed orientation):
  kvT[128, 4096] = [Wk|Wv]^T @ xT      (fused projection, full PE array)
  qT [64, 2048]  = (Wq/32)^T @ xT[:, own]
  S^T[keys,q] blocks = kT^T-slices @ qT  (f32r matmuls, causal blocks only)
  exp on ACT (PSUM->SBUF) with additive bias; staircase masks via affine_select
  outT[65, q] += [v|1]^T-tiles @ exp     (row 64 = softmax denominator, free)
  out = transpose(outT) * recip(sums), DMA to DRAM.
"""

import sys

if "/opt/trn_rl_repo" not in sys.path:
    sys.path.insert(0, "/opt/trn_rl_repo")

import numpy as np

B, T, D, DK = 4, 4096, 1024, 64
C = 1024          # T-chunk size (4 chunks per batch)
NEG = -80.0       # additive bias for masked chunks: exp(s-80) ~ 1e-35
N_CORES = 8

_CACHE = {}


def _build_nc():
    from contextlib import ExitStack

    import concourse.bass as bass  # noqa: F401
    import concourse.mybir as mybir
    import concourse.tile as tile
    from concourse import bacc
    from concourse.masks import make_identity

    f32 = mybir.dt.float32
    f32r = mybir.dt.float32r
    Exp = mybir.ActivationFunctionType.Exp

    nc = bacc.Bacc("TRN2", target_bir_lowering=False, debug=False,
                   num_devices=N_CORES)

    xt_d = nc.dram_tensor("xt", [D, T], f32, kind="ExternalInput").ap()
    wq_d = nc.dram_tensor("wq", [D, DK], f32, kind="ExternalInput").ap()
    wkv_d = nc.dram_tensor("wkv", [D, 2 * DK], f32, kind="ExternalInput").ap()
    pb_d = nc.dram_tensor("pbias", [128, 6], f32, kind="ExternalInput").ap()
    out_d = nc.dram_tensor("out", [2 * C, DK], f32, kind="ExternalOutput").ap()

    # pass table: (q-chunk sel, key layout position, diag?, bias column)
    # execution order: qA passes first (2 live outT banks), then qB.
    PASSES = [
        (0, 0, True, None),   # p0: qA vs its own chunk (diagonal)
        (0, 2, False, 1),     # p1: qA vs layout pos 2 (bias: full or off)
        (1, 0, False, 2),     # p2: qB vs pos 0
        (1, 1, True, None),   # p3: qB vs its own chunk (diagonal)
        (1, 2, False, 4),     # p4
        (1, 3, False, 5),     # p5
    ]
    # DMA / projection order of layout chunks: qA needs 0 and 2 first.
    CHUNK_ORDER = [0, 2, 1, 3]

    with tile.TileContext(nc) as tc, ExitStack() as ctx:
        wpool = ctx.enter_context(tc.tile_pool(name="w", bufs=1))
        xtpool = ctx.enter_context(tc.tile_pool(name="xt", bufs=16))
        kvpool = ctx.enter_context(tc.tile_pool(name="kv", bufs=1))
        exppool = ctx.enter_context(tc.tile_pool(name="exp", bufs=3))
        outsb = ctx.enter_context(tc.tile_pool(name="outsb", bufs=2))
        opool = ctx.enter_context(tc.tile_pool(name="o", bufs=4))
        scps = ctx.enter_context(tc.tile_pool(name="scps", bufs=2, space="PSUM"))
        prps = ctx.enter_context(tc.tile_pool(name="prps", bufs=2, space="PSUM"))

        # ---- constants / weights ----
        ident = wpool.tile([128, 128], f32)
        make_identity(nc, ident[:])
        ident_r = wpool.tile([128, 128], f32r)
        nc.vector.tensor_copy(ident_r[:], ident[:])
        wkv_s = wpool.tile([128, 8, 2 * DK], f32r)
        nc.sync.dma_start(wkv_s[:], wkv_d.rearrange("(dc p) m -> p dc m", p=128).bitcast(f32r))
        wq_s = wpool.tile([128, 8, DK], f32r)
        nc.sync.dma_start(wq_s[:], wq_d.rearrange("(dc p) m -> p dc m", p=128).bitcast(f32r))
        pb_s = wpool.tile([128, 6], f32)
        nc.sync.dma_start(pb_s[:], pb_d[:])

        # ---- persistent activations ----
        kvT = kvpool.tile([128, T], f32r)          # rows 0:64 kT, 64:128 vT
        qT = kvpool.tile([64, 2 * C], f32r)
        vall = kvpool.tile([128, 32, DK + 1], f32r)  # [keys, kt, v|1]
        ones_sc = wpool.tile([128, 32], f32)
        nc.vector.memset(ones_sc[:], 1.0)
        nc.vector.tensor_copy(
            vall[:, :, DK : DK + 1],
            ones_sc[:].rearrange("p (a b) -> p a b", b=1),
        )

        # ---- load xt + projections, chunk by chunk ----
        for lc in CHUNK_ORDER:
            xts = []
            for dc in range(8):
                t = xtpool.tile([128, C], f32r, tag="xt")
                nc.sync.dma_start(
                    t[:], xt_d[dc * 128 : (dc + 1) * 128, lc * C : (lc + 1) * C].bitcast(f32r)
                )
                xts.append(t)
            for g in range(2):  # 512-column groups within the chunk
                sl = slice(g * 512, (g + 1) * 512)
                kv_ps = prps.tile([128, 512], f32, tag="pr")
                for dc in range(8):
                    nc.tensor.matmul(
                        kv_ps[:],
                        lhsT=wkv_s[:, dc, :],
                        rhs=xts[dc][:, sl],
                        start=(dc == 0), stop=(dc == 7),
                    )
                nc.vector.tensor_copy(kvT[:, lc * C + g * 512 : lc * C + (g + 1) * 512],
                                      kv_ps[:])
                if lc < 2:  # q projection for own chunks (layout pos 0, 1)
                    q_ps = prps.tile([64, 512], f32, tag="pr")
                    for dc in range(8):
                        nc.tensor.matmul(
                            q_ps[:],
                            lhsT=wq_s[:, dc, :],
                            rhs=xts[dc][:, sl],
                            start=(dc == 0), stop=(dc == 7),
                        )
                    nc.vector.tensor_copy(
                        qT[:, lc * C + g * 512 : lc * C + (g + 1) * 512], q_ps[:]
                    )
            # v' tiles: transpose vT[64, 128] -> [128, 64] per key tile
            for ktl in range(8):
                kt = lc * 8 + ktl
                tr_ps = prps.tile([128, DK], f32r, tag="pr", name="tr_ps")
                nc.tensor.transpose(
                    tr_ps[:],
                    kvT[64:128, kt * 128 : (kt + 1) * 128],
                    ident_r[64:128, 64:128],
                )
                nc.vector.tensor_copy(vall[:, kt, 0:DK], tr_ps[:])

        # ---- attention passes ----
        # outT accumulates in SBUF (DVE adds of per-substep PSUM partials) so
        # scores PSUM can double-buffer and the PE never ping-pong-stalls on
        # ACT (which would keep HAM throttled at 1.2 GHz).
        acc = {}       # qg (0..3) -> SBUF accumulator [65, 512]

        def drain(qg):
            for j in range(4):
                tp = prps.tile([128, 65], f32, tag="pr")
                nc.tensor.transpose(
                    tp[:],
                    acc[qg][:, j * 128 : (j + 1) * 128],
                    ident[0:65, 0:65],
                )
                rec = opool.tile([128, 1], f32, tag="rec")
                nc.vector.reciprocal(rec[:], tp[:, DK : DK + 1])
                ob = opool.tile([128, DK], f32, tag="ob")
                nc.vector.tensor_scalar_mul(ob[:], tp[:, 0:DK], rec[:])
                r0 = qg * 512 + j * 128
                nc.sync.dma_start(out_d[r0 : r0 + 128, :], ob[:])

        for pi, (qsel, kp, diag, bcol) in enumerate(PASSES):
            for qg_l in range(2):
                qg = qsel * 2 + qg_l
                if qg not in acc:
                    acc[qg] = outsb.tile([65, 512], f32, tag="acc",
                                         name=f"acc{qg}")
                qc0 = qsel * 1024 + qg_l * 512
                # substeps of <=3 key tiles; diag passes skip kt>=4 for qg_l==0
                n_kt = 4 if (diag and qg_l == 0) else 8
                first_sub = pi in (0, 2)  # first pass touching this qg
                for s0 in range(0, n_kt, 3):
                    kts = list(range(s0, min(s0 + 3, n_kt)))
                    w = len(kts)
                    sc = scps.tile([128, 1536], f32, tag="sc")
                    for i, ktl in enumerate(kts):
                        kc0 = kp * 1024 + ktl * 128
                        nc.tensor.matmul(
                            sc[:, i * 512 : (i + 1) * 512],
                            lhsT=kvT[0:64, kc0 : kc0 + 128],
                            rhs=qT[:, qc0 : qc0 + 512],
                            start=True, stop=True,
                        )
                    et = exppool.tile([128, 1536], f32r, tag="et")
                    bias = pb_s[:, bcol : bcol + 1] if bcol is not None else 0.0
                    nc.scalar.activation(et[:, 0 : w * 512], sc[:, 0 : w * 512],
                                         Exp, bias=bias)
                    if diag:
                        for i, ktl in enumerate(kts):
                            m = ktl - 4 * qg_l
                            if 0 <= m < 4:
                                # keep exp where key<=q: j - p - 128m >= 0
                                nc.gpsimd.affine_select(
                                    out=et[:, i * 512 : (i + 1) * 512],
                                    in_=et[:, i * 512 : (i + 1) * 512],
                                    compare_op=mybir.AluOpType.is_ge,
                                    fill=0.0,
                                    base=-(128 * m),
                                    channel_multiplier=-1,
                                    pattern=[[1, 512]],
                                )
                    pp = prps.tile([65, 512], f32, tag="pr", name="pp")
                    for i, ktl in enumerate(kts):
                        kt = kp * 8 + ktl
                        nc.tensor.matmul(
                            pp[:],
                            lhsT=vall[:, kt, :],
                            rhs=et[:, i * 512 : (i + 1) * 512],
                            start=(i == 0), stop=(i == w - 1),
                        )
                    if first_sub and s0 == 0:
                        nc.vector.tensor_copy(acc[qg][:], pp[:])
                    else:
                        nc.vector.tensor_add(acc[qg][:], acc[qg][:], pp[:])
            if pi == 1:
                drain(0), drain(1)
            if pi == 5:
                drain(2), drain(3)

    nc.compile()
    return nc


def get_nc():
    if "nc" not in _CACHE:
        _CACHE["nc"] = _build_nc()
    return _CACHE["nc"]


def make_in_maps(x, Wq, Wk, Wv):
    wq_s = np.ascontiguousarray(Wq.astype(np.float32) / 32.0)
    wkv = np.ascontiguousarray(
        np.concatenate([Wk, Wv], axis=1).astype(np.float32)
    )
    in_maps = []
    for core in range(N_CORES):
        b, h = core // 2, core % 2
        order = [0, 3, 1, 2] if h == 0 else [1, 2, 0, 3]
        xbt = x[b].T  # [D, T] view
        xt = np.ascontiguousarray(
            np.concatenate([xbt[:, c * C : (c + 1) * C] for c in order], axis=1),
            dtype=np.float32,
        )
        bias_vals = [0, NEG, 0, 0, 0, 0] if h == 0 else [0, 0, 0, 0, 0, NEG]
        pb = np.ascontiguousarray(
            np.broadcast_to(np.array(bias_vals, np.float32), (128, 6))
        )
        in_maps.append({"xt": xt, "wq": wq_s, "wkv": wkv, "pbias": pb})
    return in_maps


def gather_out(results):
    out = np.empty((B, T, DK), np.float32)
    for core in range(N_CORES):
        b, h = core // 2, core % 2
        cA, cB = (0, 3) if h == 0 else (1, 2)
        o = results[core]["out"]
        out[b, cA * C : (cA + 1) * C] = o[0:C]
        out[b, cB * C : (cB + 1) * C] = o[C : 2 * C]
    return out


def run(in_maps, trace=False, tmpdir=None):
    from concourse.bass_utils import run_bass_kernel_spmd

    nc = get_nc()
    return run_bass_kernel_spmd(
        nc, in_maps, core_ids=list(range(N_CORES)), trace=trace, tmpdir=tmpdir
    )


def kernel(x, Wq, Wk, Wv):
    x = np.asarray(x, dtype=np.float32)
    in_maps = make_in_maps(x, np.asarray(Wq), np.asarray(Wk), np.asarray(Wv))
    res = run(in_maps)
    return gather_out(res.results)


# revision 14
# speedup vs baseline: 1.3247x; 1.2679x over previous
"""Causal single-head attention on 8 trn2 NeuronCores.

Sharding: batch b = core//2, pair-half h = core%2. Each batch's 4096 queries
split into 4 chunks of 1024; h=0 owns chunks {0,3}, h=1 owns {1,2} (balanced
causal work). Host pre-transposes x to xT[1024, T] and permutes key chunks
per-core (layout [ownA, ownB, o1, o2]) so one uniform SPMD program runs on all
cores; per-core behavior differs only through input data (chunk order + a tiny
per-pass bias table that turns never-needed key chunks off via exp(s - 80)).

Device dataflow (per core, all in transposed orientation):
  kvT[128, 4096] = [Wk|Wv]^T @ xT      (fused projection, full PE array)
  qT [64, 2048]  = (Wq/32)^T @ xT[:, own]
  S^T[keys,q] blocks = kT^T-slices @ qT  (bf16 matmuls, causal blocks only)
  exp on ACT (PSUM->SBUF) with additive bias; staircase masks via affine_select
  outT[65, q] += [v|1]^T-tiles @ exp     (row 64 = softmax denominator, free)
  out = transpose(outT) * recip(sums), DMA to DRAM.
"""

import sys

if "/opt/trn_rl_repo" not in sys.path:
    sys.path.insert(0, "/opt/trn_rl_repo")

import numpy as np

B, T, D, DK = 4, 4096, 1024, 64
C = 1024          # T-chunk size (4 chunks per batch)
NEG = -80.0       # additive bias for masked chunks: exp(s-80) ~ 1e-35
N_CORES = 8

_CACHE = {}


def _build_nc():
    from contextlib import ExitStack

    import concourse.bass as bass  # noqa: F401
    import concourse.mybir as mybir
    import concourse.tile as tile
    from concourse import bacc
    from concourse.masks import make_identity

    f32 = mybir.dt.float32
    bf16 = mybir.dt.bfloat16
    Exp = mybir.ActivationFunctionType.Exp

    nc = bacc.Bacc("TRN2", target_bir_lowering=False, debug=False,
                   num_devices=N_CORES)

    xt_d = nc.dram_tensor("xt", [D, T], bf16, kind="ExternalInput").ap()
    wq_d = nc.dram_tensor("wq", [D, DK], bf16, kind="ExternalInput").ap()
    wkv_d = nc.dram_tensor("wkv", [D, 2 * DK], bf16, kind="ExternalInput").ap()
    pb_d = nc.dram_tensor("pbias", [128, 6], f32, kind="ExternalInput").ap()
    out_d = nc.dram_tensor("out", [2 * C, DK], f32, kind="ExternalOutput").ap()

    # pass table: (q-chunk sel, key layout position, diag?, bias column)
    # execution order: qA passes first (2 live outT banks), then qB.
    PASSES = [
        (0, 0, True, None),   # p0: qA vs its own chunk (diagonal)
        (0, 2, False, 1),     # p1: qA vs layout pos 2 (bias: full or off)
        (1, 0, False, 2),     # p2: qB vs pos 0
        (1, 1, True, None),   # p3: qB vs its own chunk (diagonal)
        (1, 2, False, 4),     # p4
        (1, 3, False, 5),     # p5
    ]
    # DMA / projection order of layout chunks: qA needs 0 and 2 first.
    CHUNK_ORDER = [0, 2, 1, 3]

    with tile.TileContext(nc) as tc, ExitStack() as ctx:
        wpool = ctx.enter_context(tc.tile_pool(name="w", bufs=1))
        xtpool = ctx.enter_context(tc.tile_pool(name="xt", bufs=16))
        kvpool = ctx.enter_context(tc.tile_pool(name="kv", bufs=1))
        exppool = ctx.enter_context(tc.tile_pool(name="exp", bufs=3))
        outsb = ctx.enter_context(tc.tile_pool(name="outsb", bufs=2))
        opool = ctx.enter_context(tc.tile_pool(name="o", bufs=4))
        scps = ctx.enter_context(tc.tile_pool(name="scps", bufs=2, space="PSUM"))
        prps = ctx.enter_context(tc.tile_pool(name="prps", bufs=2, space="PSUM"))

        # ---- constants / weights ----
        ident = wpool.tile([128, 128], f32)
        make_identity(nc, ident[:])
        ident_b = wpool.tile([128, 128], bf16)
        nc.vector.tensor_copy(ident_b[:], ident[:])
        wkv_s = wpool.tile([128, 8, 2 * DK], bf16)
        nc.sync.dma_start(wkv_s[:], wkv_d.rearrange("(dc p) m -> p dc m", p=128))
        wq_s = wpool.tile([128, 8, DK], bf16)
        nc.sync.dma_start(wq_s[:], wq_d.rearrange("(dc p) m -> p dc m", p=128))
        pb_s = wpool.tile([128, 6], f32)
        nc.sync.dma_start(pb_s[:], pb_d[:])

        # ---- persistent activations ----
        kvT = kvpool.tile([128, T], bf16)          # rows 0:64 kT, 64:128 vT
        qT = kvpool.tile([64, 2 * C], bf16)
        vall = kvpool.tile([128, 32, DK + 1], bf16)  # [keys, kt, v|1]
        ones_sc = wpool.tile([128, 32], f32)
        nc.vector.memset(ones_sc[:], 1.0)
        nc.vector.tensor_copy(
            vall[:, :, DK : DK + 1],
            ones_sc[:].rearrange("p (a b) -> p a b", b=1),
        )

        # ---- load xt + projections, chunk by chunk ----
        for lc in CHUNK_ORDER:
            xts = []
            for dc in range(8):
                t = xtpool.tile([128, C], bf16, tag="xt")
                nc.sync.dma_start(
                    t[:], xt_d[dc * 128 : (dc + 1) * 128, lc * C : (lc + 1) * C]
                )
                xts.append(t)
            for g in range(2):  # 512-column groups within the chunk
                sl = slice(g * 512, (g + 1) * 512)
                kv_ps = prps.tile([128, 512], f32, tag="pr")
                for dc in range(8):
                    nc.tensor.matmul(
                        kv_ps[:],
                        lhsT=wkv_s[:, dc, :],
                        rhs=xts[dc][:, sl],
                        start=(dc == 0), stop=(dc == 7),
                    )
                nc.vector.tensor_copy(kvT[:, lc * C + g * 512 : lc * C + (g + 1) * 512],
                                      kv_ps[:])
                if lc < 2:  # q projection for own chunks (layout pos 0, 1)
                    q_ps = prps.tile([64, 512], f32, tag="pr")
                    for dc in range(8):
                        nc.tensor.matmul(
                            q_ps[:],
                            lhsT=wq_s[:, dc, :],
                            rhs=xts[dc][:, sl],
                            start=(dc == 0), stop=(dc == 7),
                        )
                    nc.vector.tensor_copy(
                        qT[:, lc * C + g * 512 : lc * C + (g + 1) * 512], q_ps[:]
                    )
            # v' tiles: transpose vT[64, 128] -> [128, 64] per key tile
            for ktl in range(8):
                kt = lc * 8 + ktl
                tr_ps = prps.tile([128, DK], bf16, tag="pr", name="tr_ps")
                nc.tensor.transpose(
                    tr_ps[:],
                    kvT[64:128, kt * 128 : (kt + 1) * 128],
                    ident_b[64:128, 64:128],
                )
                nc.vector.tensor_copy(vall[:, kt, 0:DK], tr_ps[:])

        # ---- attention passes ----
        # outT accumulates in SBUF (DVE adds of per-substep PSUM partials) so
        # scores PSUM can double-buffer and the PE never ping-pong-stalls on
        # ACT (which would keep HAM throttled at 1.2 GHz).
        acc = {}       # qg (0..3) -> SBUF accumulator [65, 512]

        def drain(qg):
            for j in range(4):
                tp = prps.tile([128, 65], f32, tag="pr")
                nc.tensor.transpose(
                    tp[:],
                    acc[qg][:, j * 128 : (j + 1) * 128],
                    ident[0:65, 0:65],
                )
                rec = opool.tile([128, 1], f32, tag="rec")
                nc.vector.reciprocal(rec[:], tp[:, DK : DK + 1])
                ob = opool.tile([128, DK], f32, tag="ob")
                nc.vector.tensor_scalar_mul(ob[:], tp[:, 0:DK], rec[:])
                r0 = qg * 512 + j * 128
                nc.sync.dma_start(out_d[r0 : r0 + 128, :], ob[:])

        for pi, (qsel, kp, diag, bcol) in enumerate(PASSES):
            for qg_l in range(2):
                qg = qsel * 2 + qg_l
                if qg not in acc:
                    acc[qg] = outsb.tile([65, 512], f32, tag="acc",
                                         name=f"acc{qg}")
                qc0 = qsel * 1024 + qg_l * 512
                # substeps of <=3 key tiles; diag passes skip kt>=4 for qg_l==0
                n_kt = 4 if (diag and qg_l == 0) else 8
                first_sub = pi in (0, 2)  # first pass touching this qg
                for s0 in range(0, n_kt, 3):
                    kts = list(range(s0, min(s0 + 3, n_kt)))
                    w = len(kts)
                    sc = scps.tile([128, 1536], f32, tag="sc")
                    for i, ktl in enumerate(kts):
                        kc0 = kp * 1024 + ktl * 128
                        nc.tensor.matmul(
                            sc[:, i * 512 : (i + 1) * 512],
                            lhsT=kvT[0:64, kc0 : kc0 + 128],
                            rhs=qT[:, qc0 : qc0 + 512],
                            start=True, stop=True,
                        )
                    et = exppool.tile([128, 1536], bf16, tag="et")
                    bias = pb_s[:, bcol : bcol + 1] if bcol is not None else 0.0
                    nc.scalar.activation(et[:, 0 : w * 512], sc[:, 0 : w * 512],
                                         Exp, bias=bias)
                    if diag:
                        for i, ktl in enumerate(kts):
                            m = ktl - 4 * qg_l
                            if 0 <= m < 4:
                                # keep exp where key<=q: j - p - 128m >= 0
                                nc.gpsimd.affine_select(
                                    out=et[:, i * 512 : (i + 1) * 512],
                                    in_=et[:, i * 512 : (i + 1) * 512],
                                    compare_op=mybir.AluOpType.is_ge,
                                    fill=0.0,
                                    base=-(128 * m),
                                    channel_multiplier=-1,
                                    pattern=[[1, 512]],
                                )
                    pp = prps.tile([65, 512], f32, tag="pr", name="pp")
                    for i, ktl in enumerate(kts):
                        kt = kp * 8 + ktl
                        nc.tensor.matmul(
                            pp[:],
                            lhsT=vall[:, kt, :],
                            rhs=et[:, i * 512 : (i + 1) * 512],
                            start=(i == 0), stop=(i == w - 1),
                        )
                    if first_sub and s0 == 0:
                        nc.vector.tensor_copy(acc[qg][:], pp[:])
                    else:
                        nc.vector.tensor_add(acc[qg][:], acc[qg][:], pp[:])
            if pi == 1:
                drain(0), drain(1)
            if pi == 5:
                drain(2), drain(3)

    nc.compile()
    return nc


def get_nc():
    if "nc" not in _CACHE:
        _CACHE["nc"] = _build_nc()
    return _CACHE["nc"]


def make_in_maps(x, Wq, Wk, Wv):
    import ml_dtypes

    bf = ml_dtypes.bfloat16
    wq_s = np.ascontiguousarray((np.asarray(Wq, np.float32) / 32.0).astype(bf))
    wkv = np.ascontiguousarray(
        np.concatenate([Wk, Wv], axis=1).astype(np.float32).astype(bf)
    )
    in_maps = []
    for core in range(N_CORES):
        b, h = core // 2, core % 2
        order = [0, 3, 1, 2] if h == 0 else [1, 2, 0, 3]
        xbt = x[b].T  # [D, T] view
        xt = np.ascontiguousarray(
            np.concatenate([xbt[:, c * C : (c + 1) * C] for c in order], axis=1)
        ).astype(bf)
        bias_vals = [0, NEG, 0, 0, 0, 0] if h == 0 else [0, 0, 0, 0, 0, NEG]
        pb = np.ascontiguousarray(
            np.broadcast_to(np.array(bias_vals, np.float32), (128, 6))
        )
        in_maps.append({"xt": xt, "wq": wq_s, "wkv": wkv, "pbias": pb})
    return in_maps


def gather_out(results):
    out = np.empty((B, T, DK), np.float32)
    for core in range(N_CORES):
        b, h = core // 2, core % 2
        cA, cB = (0, 3) if h == 0 else (1, 2)
        o = results[core]["out"]
        out[b, cA * C : (cA + 1) * C] = o[0:C]
        out[b, cB * C : (cB + 1) * C] = o[C : 2 * C]
    return out


def run(in_maps, trace=False, tmpdir=None):
    from concourse.bass_utils import run_bass_kernel_spmd

    nc = get_nc()
    return run_bass_kernel_spmd(
        nc, in_maps, core_ids=list(range(N_CORES)), trace=trace, tmpdir=tmpdir
    )


def kernel(x, Wq, Wk, Wv):
    x = np.asarray(x, dtype=np.float32)
    in_maps = make_in_maps(x, np.asarray(Wq), np.asarray(Wk), np.asarray(Wv))
    res = run(in_maps)
    return gather_out(res.results)


# revision 15
# speedup vs baseline: 1.4421x; 1.0886x over previous
"""Causal single-head attention on 8 trn2 NeuronCores.

Sharding: batch b = core//2, pair-half h = core%2. Each batch's 4096 queries
split into 4 chunks of 1024; h=0 owns chunks {0,3}, h=1 owns {1,2} (balanced
causal work). Host pre-transposes x to xT[1024, T] and permutes key chunks
per-core (layout [ownA, ownB, o1, o2]) so one uniform SPMD program runs on all
cores; per-core behavior differs only through input data (chunk order + a tiny
per-pass bias table that turns never-needed key chunks off via exp(s - 80)).

Device dataflow (per core, all in transposed orientation):
  kvT[128, 4096] = [Wk|Wv]^T @ xT      (fused projection, full PE array)
  qT [64, 2048]  = (Wq/32)^T @ xT[:, own]
  S^T[keys,q] blocks = kT^T-slices @ qT  (bf16 matmuls, causal blocks only)
  exp on ACT (PSUM->SBUF) with additive bias; staircase masks via affine_select
  outT[65, q] += [v|1]^T-tiles @ exp     (row 64 = softmax denominator, free)
  out = transpose(outT) * recip(sums), DMA to DRAM.
"""

import sys

if "/opt/trn_rl_repo" not in sys.path:
    sys.path.insert(0, "/opt/trn_rl_repo")

import numpy as np

B, T, D, DK = 4, 4096, 1024, 64
C = 1024          # T-chunk size (4 chunks per batch)
NEG = -80.0       # additive bias for masked chunks: exp(s-80) ~ 1e-35
N_CORES = 8

_CACHE = {}


def _build_nc():
    from contextlib import ExitStack

    import concourse.bass as bass  # noqa: F401
    import concourse.mybir as mybir
    import concourse.tile as tile
    from concourse import bacc
    from concourse.masks import make_identity

    f32 = mybir.dt.float32
    bf16 = mybir.dt.bfloat16
    Exp = mybir.ActivationFunctionType.Exp

    nc = bacc.Bacc("TRN2", target_bir_lowering=False, debug=False,
                   num_devices=N_CORES)

    xt_d = nc.dram_tensor("xt", [D, T], bf16, kind="ExternalInput").ap()
    wq_d = nc.dram_tensor("wq", [D, DK], bf16, kind="ExternalInput").ap()
    wkv_d = nc.dram_tensor("wkv", [D, 2 * DK], bf16, kind="ExternalInput").ap()
    pb_d = nc.dram_tensor("pbias", [128, 6], f32, kind="ExternalInput").ap()
    out_d = nc.dram_tensor("out", [2 * C, DK], f32, kind="ExternalOutput").ap()

    # pass table: (q-chunk sel, key layout position, diag?, bias column)
    # execution order: qA passes first (2 live outT banks), then qB.
    PASSES = [
        (0, 0, True, None),   # p0: qA vs its own chunk (diagonal)
        (0, 2, False, 1),     # p1: qA vs layout pos 2 (bias: full or off)
        (1, 0, False, 2),     # p2: qB vs pos 0
        (1, 1, True, None),   # p3: qB vs its own chunk (diagonal)
        (1, 2, False, 4),     # p4
        (1, 3, False, 5),     # p5
    ]
    # DMA / projection order of layout chunks: qA needs 0 and 2 first.
    CHUNK_ORDER = [0, 2, 1, 3]

    with tile.TileContext(nc) as tc, ExitStack() as ctx:
        wpool = ctx.enter_context(tc.tile_pool(name="w", bufs=1))
        xtpool = ctx.enter_context(tc.tile_pool(name="xt", bufs=16))
        kvpool = ctx.enter_context(tc.tile_pool(name="kv", bufs=1))
        exppool = ctx.enter_context(tc.tile_pool(name="exp", bufs=3))
        outsb = ctx.enter_context(tc.tile_pool(name="outsb", bufs=2))
        opool = ctx.enter_context(tc.tile_pool(name="o", bufs=4))
        scps = ctx.enter_context(tc.tile_pool(name="scps", bufs=2, space="PSUM"))
        prps = ctx.enter_context(tc.tile_pool(name="prps", bufs=2, space="PSUM"))

        # ---- constants / weights ----
        ident = wpool.tile([128, 128], f32)
        make_identity(nc, ident[:])
        ident_b = wpool.tile([128, 128], bf16)
        nc.vector.tensor_copy(ident_b[:], ident[:])
        wkv_s = wpool.tile([128, 8, 2 * DK], bf16)
        nc.sync.dma_start(wkv_s[:], wkv_d.rearrange("(dc p) m -> p dc m", p=128))
        wq_s = wpool.tile([128, 8, DK], bf16)
        nc.sync.dma_start(wq_s[:], wq_d.rearrange("(dc p) m -> p dc m", p=128))
        pb_s = wpool.tile([128, 6], f32)
        nc.sync.dma_start(pb_s[:], pb_d[:])

        # ---- persistent activations ----
        kvT = kvpool.tile([128, T], bf16)          # rows 0:64 kT, 64:128 vT
        qT = kvpool.tile([64, 2 * C], bf16)
        vall = kvpool.tile([128, 32, DK + 1], bf16)  # [keys, kt, v|1]
        ones_sc = wpool.tile([128, 32], f32)
        nc.vector.memset(ones_sc[:], 1.0)
        nc.vector.tensor_copy(
            vall[:, :, DK : DK + 1],
            ones_sc[:].rearrange("p (a b) -> p a b", b=1),
        )

        # ---- load xt + projections, chunk by chunk ----
        for lc in CHUNK_ORDER:
            xts = []
            for dc in range(8):
                t = xtpool.tile([128, C], bf16, tag="xt")
                nc.sync.dma_start(
                    t[:], xt_d[dc * 128 : (dc + 1) * 128, lc * C : (lc + 1) * C]
                )
                xts.append(t)
            for g in range(2):  # 512-column groups within the chunk
                sl = slice(g * 512, (g + 1) * 512)
                kv_ps = prps.tile([128, 512], f32, tag="pr")
                for dc in range(8):
                    nc.tensor.matmul(
                        kv_ps[:],
                        lhsT=wkv_s[:, dc, :],
                        rhs=xts[dc][:, sl],
                        start=(dc == 0), stop=(dc == 7),
                    )
                nc.vector.tensor_copy(kvT[:, lc * C + g * 512 : lc * C + (g + 1) * 512],
                                      kv_ps[:])
                if lc < 2:  # q projection for own chunks (layout pos 0, 1)
                    q_ps = prps.tile([64, 512], f32, tag="pr")
                    for dc in range(8):
                        nc.tensor.matmul(
                            q_ps[:],
                            lhsT=wq_s[:, dc, :],
                            rhs=xts[dc][:, sl],
                            start=(dc == 0), stop=(dc == 7),
                        )
                    nc.vector.tensor_copy(
                        qT[:, lc * C + g * 512 : lc * C + (g + 1) * 512], q_ps[:]
                    )
            # v' tiles: transpose vT[64, 128] -> [128, 64] per key tile
            for ktl in range(8):
                kt = lc * 8 + ktl
                tr_ps = prps.tile([128, DK], bf16, tag="pr", name="tr_ps")
                nc.tensor.transpose(
                    tr_ps[:],
                    kvT[64:128, kt * 128 : (kt + 1) * 128],
                    ident_b[64:128, 64:128],
                )
                nc.vector.tensor_copy(vall[:, kt, 0:DK], tr_ps[:])

        # ---- attention passes ----
        # outT accumulates in SBUF (DVE adds of per-substep PSUM partials) so
        # scores PSUM can double-buffer and the PE never ping-pong-stalls on
        # ACT (which would keep HAM throttled at 1.2 GHz).
        acc = {}       # qg (0..3) -> SBUF accumulator [65, 512]

        def drain(qg):
            for j in range(4):
                tp = prps.tile([128, 65], f32, tag="pr")
                nc.tensor.transpose(
                    tp[:],
                    acc[qg][:, j * 128 : (j + 1) * 128],
                    ident[0:65, 0:65],
                )
                rec = opool.tile([128, 1], f32, tag="rec")
                nc.vector.reciprocal(rec[:], tp[:, DK : DK + 1])
                ob = opool.tile([128, DK], f32, tag="ob")
                nc.vector.tensor_scalar_mul(ob[:], tp[:, 0:DK], rec[:])
                r0 = qg * 512 + j * 128
                nc.sync.dma_start(out_d[r0 : r0 + 128, :], ob[:])

        # flat substep list, then 1-deep software pipeline: emit scores(s+1)
        # before outT(s) so the PE never stalls waiting for ACT's exp(s).
        steps = []
        for pi, (qsel, kp, diag, bcol) in enumerate(PASSES):
            for qg_l in range(2):
                qg = qsel * 2 + qg_l
                qc0 = qsel * 1024 + qg_l * 512
                n_kt = 4 if (diag and qg_l == 0) else 8
                for s0 in range(0, n_kt, 3):
                    kts = list(range(s0, min(s0 + 3, n_kt)))
                    first = pi in (0, 2) and s0 == 0
                    steps.append(dict(pi=pi, qg=qg, qc0=qc0, kp=kp, diag=diag,
                                      bcol=bcol, qg_l=qg_l, kts=kts,
                                      first=first))
        last_of_pass = {}
        for idx, st in enumerate(steps):
            last_of_pass[st["pi"]] = idx

        def emit_scores(st):
            kts, w = st["kts"], len(st["kts"])
            sc = scps.tile([128, 1536], f32, tag="sc", name="sc")
            for i, ktl in enumerate(kts):
                kc0 = st["kp"] * 1024 + ktl * 128
                nc.tensor.matmul(
                    sc[:, i * 512 : (i + 1) * 512],
                    lhsT=kvT[0:64, kc0 : kc0 + 128],
                    rhs=qT[:, st["qc0"] : st["qc0"] + 512],
                    start=True, stop=True,
                )
            et = exppool.tile([128, 1536], bf16, tag="et", name="et")
            bcol = st["bcol"]
            bias = pb_s[:, bcol : bcol + 1] if bcol is not None else 0.0
            nc.scalar.activation(et[:, 0 : w * 512], sc[:, 0 : w * 512],
                                 Exp, bias=bias)
            if st["diag"]:
                for i, ktl in enumerate(kts):
                    m = ktl - 4 * st["qg_l"]
                    if 0 <= m < 4:
                        # keep exp where key<=q: j - p - 128m >= 0
                        nc.gpsimd.affine_select(
                            out=et[:, i * 512 : (i + 1) * 512],
                            in_=et[:, i * 512 : (i + 1) * 512],
                            compare_op=mybir.AluOpType.is_ge,
                            fill=0.0,
                            base=-(128 * m),
                            channel_multiplier=-1,
                            pattern=[[1, 512]],
                        )
            st["et"] = et

        def emit_outT(st, idx):
            kts, w, qg = st["kts"], len(st["kts"]), st["qg"]
            et = st["et"]
            if qg not in acc:
                acc[qg] = outsb.tile([65, 512], f32, tag="acc",
                                     name=f"acc{qg}")
            pp = prps.tile([65, 512], f32, tag="pr", name="pp")
            for i, ktl in enumerate(kts):
                kt = st["kp"] * 8 + ktl
                nc.tensor.matmul(
                    pp[:],
                    lhsT=vall[:, kt, :],
                    rhs=et[:, i * 512 : (i + 1) * 512],
                    start=(i == 0), stop=(i == w - 1),
                )
            if st["first"]:
                nc.vector.tensor_copy(acc[qg][:], pp[:])
            else:
                nc.vector.tensor_add(acc[qg][:], acc[qg][:], pp[:])
            if idx == last_of_pass[1]:
                drain(0), drain(1)
            if idx == last_of_pass[5]:
                drain(2), drain(3)

        pending = None
        for idx, st in enumerate(steps):
            emit_scores(st)
            if pending is not None:
                emit_outT(*pending)
            pending = (st, idx)
        emit_outT(*pending)

    nc.compile()
    return nc


def get_nc():
    if "nc" not in _CACHE:
        _CACHE["nc"] = _build_nc()
    return _CACHE["nc"]


def make_in_maps(x, Wq, Wk, Wv):
    import ml_dtypes

    bf = ml_dtypes.bfloat16
    wq_s = np.ascontiguousarray((np.asarray(Wq, np.float32) / 32.0).astype(bf))
    wkv = np.ascontiguousarray(
        np.concatenate([Wk, Wv], axis=1).astype(np.float32).astype(bf)
    )
    in_maps = []
    for core in range(N_CORES):
        b, h = core // 2, core % 2
        order = [0, 3, 1, 2] if h == 0 else [1, 2, 0, 3]
        xbt = x[b].T  # [D, T] view
        xt = np.ascontiguousarray(
            np.concatenate([xbt[:, c * C : (c + 1) * C] for c in order], axis=1)
        ).astype(bf)
        bias_vals = [0, NEG, 0, 0, 0, 0] if h == 0 else [0, 0, 0, 0, 0, NEG]
        pb = np.ascontiguousarray(
            np.broadcast_to(np.array(bias_vals, np.float32), (128, 6))
        )
        in_maps.append({"xt": xt, "wq": wq_s, "wkv": wkv, "pbias": pb})
    return in_maps


def gather_out(results):
    out = np.empty((B, T, DK), np.float32)
    for core in range(N_CORES):
        b, h = core // 2, core % 2
        cA, cB = (0, 3) if h == 0 else (1, 2)
        o = results[core]["out"]
        out[b, cA * C : (cA + 1) * C] = o[0:C]
        out[b, cB * C : (cB + 1) * C] = o[C : 2 * C]
    return out


def run(in_maps, trace=False, tmpdir=None):
    from concourse.bass_utils import run_bass_kernel_spmd

    nc = get_nc()
    return run_bass_kernel_spmd(
        nc, in_maps, core_ids=list(range(N_CORES)), trace=trace, tmpdir=tmpdir
    )


def kernel(x, Wq, Wk, Wv):
    x = np.asarray(x, dtype=np.float32)
    in_maps = make_in_maps(x, np.asarray(Wq), np.asarray(Wk), np.asarray(Wv))
    res = run(in_maps)
    return gather_out(res.results)


# revision 17
# speedup vs baseline: 1.4988x; 1.0393x over previous
"""Causal single-head attention on 8 trn2 NeuronCores.

Sharding: batch b = core//2, pair-half h = core%2. Each batch's 4096 queries
split into 4 chunks of 1024; h=0 owns chunks {0,3}, h=1 owns {1,2} (balanced
causal work). Host pre-transposes x to xT[1024, T] and permutes key chunks
per-core (layout [ownA, ownB, o1, o2]) so one uniform SPMD program runs on all
cores; per-core behavior differs only through input data (chunk order + a tiny
per-pass bias table that turns never-needed key chunks off via exp(s - 80)).

Device dataflow (per core, all in transposed orientation):
  kvT[128, 4096] = [Wk|Wv]^T @ xT      (fused projection, full PE array)
  qT [64, 2048]  = (Wq/32)^T @ xT[:, own]
  S^T[keys,q] blocks = kT^T-slices @ qT  (bf16 matmuls, causal blocks only)
  exp on ACT (PSUM->SBUF) with additive bias; staircase masks via affine_select
  outT[65, q] += [v|1]^T-tiles @ exp     (row 64 = softmax denominator, free)
  out = transpose(outT) * recip(sums), DMA to DRAM.
"""

import sys

if "/opt/trn_rl_repo" not in sys.path:
    sys.path.insert(0, "/opt/trn_rl_repo")

import numpy as np

B, T, D, DK = 4, 4096, 1024, 64
C = 1024          # T-chunk size (4 chunks per batch)
NEG = -80.0       # additive bias for masked chunks: exp(s-80) ~ 1e-35
N_CORES = 8

_CACHE = {}


def _build_nc():
    from contextlib import ExitStack

    import concourse.bass as bass  # noqa: F401
    import concourse.mybir as mybir
    import concourse.tile as tile
    from concourse import bacc
    from concourse.masks import make_identity

    f32 = mybir.dt.float32
    bf16 = mybir.dt.bfloat16
    Exp = mybir.ActivationFunctionType.Exp

    nc = bacc.Bacc("TRN2", target_bir_lowering=False, debug=False,
                   num_devices=N_CORES)

    xt_d = nc.dram_tensor("xt", [D, T], bf16, kind="ExternalInput").ap()
    wq_d = nc.dram_tensor("wq", [D, DK], bf16, kind="ExternalInput").ap()
    wkv_d = nc.dram_tensor("wkv", [D, 2 * DK], bf16, kind="ExternalInput").ap()
    pb_d = nc.dram_tensor("pbias", [128, 6], f32, kind="ExternalInput").ap()
    out_d = nc.dram_tensor("out", [2 * C, DK], f32, kind="ExternalOutput").ap()

    # pass table: (q-chunk sel, key layout position, diag?, bias column)
    # layout per core: [ownA, other1, ownB, other2] -> sequential DMA order,
    # qA passes need only positions 0-1, qB's diag is structurally pos 2.
    PASSES = [
        (0, 0, True, None),   # p0: qA vs its own chunk (diagonal)
        (0, 1, False, 1),     # p1: qA vs layout pos 1 (bias: full or off)
        (1, 0, False, 2),     # p2: qB vs pos 0
        (1, 1, False, 3),     # p3: qB vs pos 1
        (1, 2, True, None),   # p4: qB vs its own chunk (diagonal)
        (1, 3, False, 5),     # p5: qB vs pos 3
    ]
    CHUNK_ORDER = [0, 1, 2, 3]

    with tile.TileContext(nc) as tc, ExitStack() as ctx:
        wpool = ctx.enter_context(tc.tile_pool(name="w", bufs=1))
        xtpool = ctx.enter_context(tc.tile_pool(name="xt", bufs=16))
        kvpool = ctx.enter_context(tc.tile_pool(name="kv", bufs=1))
        exppool = ctx.enter_context(tc.tile_pool(name="exp", bufs=3))
        outsb = ctx.enter_context(tc.tile_pool(name="outsb", bufs=2))
        opool = ctx.enter_context(tc.tile_pool(name="o", bufs=4))
        scps = ctx.enter_context(tc.tile_pool(name="scps", bufs=2, space="PSUM"))
        prps = ctx.enter_context(tc.tile_pool(name="prps", bufs=2, space="PSUM"))

        # ---- constants / weights ----
        ident = wpool.tile([128, 128], f32)
        make_identity(nc, ident[:])
        ident_b = wpool.tile([128, 128], bf16)
        nc.vector.tensor_copy(ident_b[:], ident[:])
        wkv_s = wpool.tile([128, 8, 2 * DK], bf16)
        nc.sync.dma_start(wkv_s[:], wkv_d.rearrange("(dc p) m -> p dc m", p=128))
        wq_s = wpool.tile([128, 8, DK], bf16)
        nc.sync.dma_start(wq_s[:], wq_d.rearrange("(dc p) m -> p dc m", p=128))
        pb_s = wpool.tile([128, 6], f32)
        nc.sync.dma_start(pb_s[:], pb_d[:])

        # ---- persistent activations ----
        # ktt/qtt hold kT/qT twice (rows 0:64 and 64:128) so scores matmuls
        # can pack two K=64 tiles into disjoint PE row groups concurrently.
        ktt = kvpool.tile([128, T], bf16)
        qtt = kvpool.tile([128, 2 * C], bf16)
        vtile = kvpool.tile([128, T], bf16)        # rows 64:128 hold vT
        vall = kvpool.tile([128, 32, DK + 1], bf16)  # [keys, kt, v|1]
        ones_sc = wpool.tile([128, 32], f32)
        nc.vector.memset(ones_sc[:], 1.0)
        nc.vector.tensor_copy(
            vall[:, :, DK : DK + 1],
            ones_sc[:].rearrange("p (a b) -> p a b", b=1),
        )

        # ---- load xt + projections, chunk by chunk ----
        for lc in CHUNK_ORDER:
            xts = []
            for dc in range(8):
                t = xtpool.tile([128, C], bf16, tag="xt")
                nc.sync.dma_start(
                    t[:], xt_d[dc * 128 : (dc + 1) * 128, lc * C : (lc + 1) * C]
                )
                xts.append(t)
            qcol = {0: 0, 2: C}.get(lc)  # own chunks sit at layout pos 0, 2
            for g in range(2):  # 512-column groups within the chunk
                sl = slice(g * 512, (g + 1) * 512)
                cs = slice(lc * C + g * 512, lc * C + (g + 1) * 512)
                kv_ps = prps.tile([128, 512], f32, tag="pr")
                for dc in range(8):
                    nc.tensor.matmul(
                        kv_ps[:],
                        lhsT=wkv_s[:, dc, :],
                        rhs=xts[dc][:, sl],
                        start=(dc == 0), stop=(dc == 7),
                    )
                nc.vector.tensor_copy(ktt[0:64, cs], kv_ps[0:64, :])
                nc.vector.tensor_copy(vtile[64:128, cs], kv_ps[64:128, :])
                if qcol is not None:
                    q_ps = prps.tile([64, 512], f32, tag="pr")
                    for dc in range(8):
                        nc.tensor.matmul(
                            q_ps[:],
                            lhsT=wq_s[:, dc, :],
                            rhs=xts[dc][:, sl],
                            start=(dc == 0), stop=(dc == 7),
                        )
                    nc.vector.tensor_copy(
                        qtt[0:64, qcol + g * 512 : qcol + (g + 1) * 512], q_ps[:]
                    )
            # duplicate kT (and qT) into partitions 64:128 via SBUF->SBUF DMA
            nc.sync.dma_start(ktt[64:128, lc * C : (lc + 1) * C],
                              ktt[0:64, lc * C : (lc + 1) * C])
            if qcol is not None:
                nc.sync.dma_start(qtt[64:128, qcol : qcol + C],
                                  qtt[0:64, qcol : qcol + C])
            # v' tiles: transpose vT[64, 128] -> [128, 64] per key tile
            for ktl in range(8):
                kt = lc * 8 + ktl
                tr_ps = prps.tile([128, DK], bf16, tag="pr", name="tr_ps")
                nc.tensor.transpose(
                    tr_ps[:],
                    vtile[64:128, kt * 128 : (kt + 1) * 128],
                    ident_b[64:128, 64:128],
                )
                nc.vector.tensor_copy(vall[:, kt, 0:DK], tr_ps[:])

        # ---- attention passes ----
        # outT accumulates in SBUF (DVE adds of per-substep PSUM partials) so
        # scores PSUM can double-buffer and the PE never ping-pong-stalls on
        # ACT (which would keep HAM throttled at 1.2 GHz).
        acc = {}       # qg (0..3) -> SBUF accumulator [65, 512]

        def drain(qg):
            for j in range(4):
                tp = prps.tile([128, 65], f32, tag="pr")
                nc.tensor.transpose(
                    tp[:],
                    acc[qg][:, j * 128 : (j + 1) * 128],
                    ident[0:65, 0:65],
                )
                rec = opool.tile([128, 1], f32, tag="rec")
                nc.vector.reciprocal(rec[:], tp[:, DK : DK + 1])
                ob = opool.tile([128, DK], f32, tag="ob")
                nc.vector.tensor_scalar_mul(ob[:], tp[:, 0:DK], rec[:])
                r0 = qg * 512 + j * 128
                nc.sync.dma_start(out_d[r0 : r0 + 128, :], ob[:])

        # flat substep list, then 1-deep software pipeline: emit scores(s+1)
        # before outT(s) so the PE never stalls waiting for ACT's exp(s).
        steps = []
        for pi, (qsel, kp, diag, bcol) in enumerate(PASSES):
            for qg_l in range(2):
                qg = qsel * 2 + qg_l
                qc0 = qsel * 1024 + qg_l * 512
                n_kt = 4 if (diag and qg_l == 0) else 8
                for s0 in range(0, n_kt, 3):
                    kts = list(range(s0, min(s0 + 3, n_kt)))
                    first = pi in (0, 2) and s0 == 0
                    steps.append(dict(pi=pi, qg=qg, qc0=qc0, kp=kp, diag=diag,
                                      bcol=bcol, qg_l=qg_l, kts=kts,
                                      first=first))
        last_of_pass = {}
        for idx, st in enumerate(steps):
            last_of_pass[st["pi"]] = idx

        def emit_scores(st):
            kts, w = st["kts"], len(st["kts"])
            qc0 = st["qc0"]
            sc = scps.tile([128, 1536], f32, tag="sc", name="sc")
            # kt pairs go to PE row groups (0,64) and run concurrently
            for i, ktl in enumerate(kts):
                kc0 = st["kp"] * 1024 + ktl * 128
                half = (i % 2) * 64
                nc.tensor.matmul(
                    sc[:, i * 512 : (i + 1) * 512],
                    lhsT=ktt[half : half + 64, kc0 : kc0 + 128],
                    rhs=qtt[half : half + 64, qc0 : qc0 + 512],
                    start=True, stop=True,
                )
            et = exppool.tile([128, 1536], bf16, tag="et", name="et")
            bcol = st["bcol"]
            bias = pb_s[:, bcol : bcol + 1] if bcol is not None else 0.0
            nc.scalar.activation(et[:, 0 : w * 512], sc[:, 0 : w * 512],
                                 Exp, bias=bias)
            if st["diag"]:
                for i, ktl in enumerate(kts):
                    m = ktl - 4 * st["qg_l"]
                    if 0 <= m < 4:
                        # keep exp where key<=q: j - p - 128m >= 0
                        nc.gpsimd.affine_select(
                            out=et[:, i * 512 : (i + 1) * 512],
                            in_=et[:, i * 512 : (i + 1) * 512],
                            compare_op=mybir.AluOpType.is_ge,
                            fill=0.0,
                            base=-(128 * m),
                            channel_multiplier=-1,
                            pattern=[[1, 512]],
                        )
            st["et"] = et

        def emit_outT(st, idx):
            kts, w, qg = st["kts"], len(st["kts"]), st["qg"]
            et = st["et"]
            if qg not in acc:
                acc[qg] = outsb.tile([65, 512], f32, tag="acc",
                                     name=f"acc{qg}")
            pp = prps.tile([65, 512], f32, tag="pr", name="pp")
            for i, ktl in enumerate(kts):
                kt = st["kp"] * 8 + ktl
                nc.tensor.matmul(
                    pp[:],
                    lhsT=vall[:, kt, :],
                    rhs=et[:, i * 512 : (i + 1) * 512],
                    start=(i == 0), stop=(i == w - 1),
                )
            if st["first"]:
                nc.vector.tensor_copy(acc[qg][:], pp[:])
            else:
                nc.vector.tensor_add(acc[qg][:], acc[qg][:], pp[:])
            if idx == last_of_pass[1]:
                drain(0), drain(1)
            if idx == last_of_pass[5]:
                drain(2), drain(3)

        pending = None
        for idx, st in enumerate(steps):
            emit_scores(st)
            if pending is not None:
                emit_outT(*pending)
            pending = (st, idx)
        emit_outT(*pending)

    nc.compile()
    return nc


def get_nc():
    if "nc" not in _CACHE:
        _CACHE["nc"] = _build_nc()
    return _CACHE["nc"]


def make_in_maps(x, Wq, Wk, Wv):
    import ml_dtypes

    bf = ml_dtypes.bfloat16
    wq_s = np.ascontiguousarray((np.asarray(Wq, np.float32) / 32.0).astype(bf))
    wkv = np.ascontiguousarray(
        np.concatenate([Wk, Wv], axis=1).astype(np.float32).astype(bf)
    )
    in_maps = []
    for core in range(N_CORES):
        b, h = core // 2, core % 2
        order = [0, 1, 3, 2] if h == 0 else [1, 0, 2, 3]
        xbt = x[b].T  # [D, T] view
        xt = np.ascontiguousarray(
            np.concatenate([xbt[:, c * C : (c + 1) * C] for c in order], axis=1)
        ).astype(bf)
        bias_vals = [0, NEG, 0, 0, 0, 0] if h == 0 else [0, 0, 0, 0, 0, NEG]
        pb = np.ascontiguousarray(
            np.broadcast_to(np.array(bias_vals, np.float32), (128, 6))
        )
        in_maps.append({"xt": xt, "wq": wq_s, "wkv": wkv, "pbias": pb})
    return in_maps


def gather_out(results):
    out = np.empty((B, T, DK), np.float32)
    for core in range(N_CORES):
        b, h = core // 2, core % 2
        cA, cB = (0, 3) if h == 0 else (1, 2)
        o = results[core]["out"]
        out[b, cA * C : (cA + 1) * C] = o[0:C]
        out[b, cB * C : (cB + 1) * C] = o[C : 2 * C]
    return out


def run(in_maps, trace=False, tmpdir=None):
    from concourse.bass_utils import run_bass_kernel_spmd

    nc = get_nc()
    return run_bass_kernel_spmd(
        nc, in_maps, core_ids=list(range(N_CORES)), trace=trace, tmpdir=tmpdir
    )


def kernel(x, Wq, Wk, Wv):
    x = np.asarray(x, dtype=np.float32)
    in_maps = make_in_maps(x, np.asarray(Wq), np.asarray(Wk), np.asarray(Wv))
    res = run(in_maps)
    return gather_out(res.results)
